# revision 1
# baseline (speedup 1.0000x reference)
"""DNDT forward kernel for Trainium2 (8 NeuronCores, data-parallel).

Math (matches the reference):
    w = [1,2,3,4];  b = [0, cumsum(-sort(beta))]
    sigma[i,f,k] = sigmoid((x[i,f]*w[k] + b[k]) / T)            [B, 6, 4]
    leaves[i]    = kron(sigma[i,0], ..., sigma[i,5])            [B, 4096]
    out          = leaves @ L                                   [B, 10]

Restructured to avoid materializing the 4096-wide leaves:
    A[i,a]  = kron(s0, s1)          a = k0*4+k1      in [0,16)
    Bm[i,b] = kron(s2, s3, s4, s5)  b = k2*64+...+k5 in [0,256)
    M[i,(a,c)] = sum_b Bm[i,b] * L3[b, (a,c)]   (PE matmul, K=256 in 2 chunks)
        where L3[b, a*10+c] = L[a*256+b, c]
    out[i,c] = sum_a A[i,a] * M[i,(a,c)]        (DVE multiply + strided reduce)

Per-core layout: 8192 rows, processed as 8 supertiles of 1024 rows.
Within a supertile, partition p holds rows {base + p*G + q : q in [0,G)}.
For each q, Bm[:, q, :] is a [128,256] row-major block; two PE transposes
produce the [256,128] lhsT for the matmul.
"""

import numpy as np

import concourse.bacc as bacc
import concourse.mybir as mybir
import concourse.tile as tile
from concourse.bass_utils import run_bass_kernel_spmd

F32 = mybir.dt.float32
F16 = mybir.dt.float16

B, F, NB, NCLS = 65536, 6, 4, 10
CORES = 8
ROWS = B // CORES          # 8192 rows per core
G = 8                      # row-groups (matmul tiles) per supertile
ST_ROWS = 128 * G          # 1024 rows per supertile
N_ST = ROWS // ST_ROWS     # 8 supertiles
TEMP = 0.1

_NC_CACHE = {}

import os
ACC_MODE = os.environ.get("K_ACC", "fast")   # "fast" | "mid" | "acc"


def _build_nc():
    nc = bacc.Bacc("TRN2", target_bir_lowering=False, debug=False)

    xc = nc.dram_tensor("xc", [ROWS, F], F32, kind="ExternalInput")
    wt = nc.dram_tensor("wt", [128, 24], F32, kind="ExternalInput")
    bt = nc.dram_tensor("bt", [128, 24], F32, kind="ExternalInput")
    ident = nc.dram_tensor("ident", [128, 128], F16, kind="ExternalInput")
    l3p = nc.dram_tensor("l3p", [128, 2, 160], F16, kind="ExternalInput")
    outc = nc.dram_tensor("outc", [ROWS, NCLS], F32, kind="ExternalOutput")

    with tile.TileContext(nc) as tc:
        with (
            tc.tile_pool(name="consts", bufs=1) as consts,
            tc.tile_pool(name="io", bufs=4) as io,
            tc.tile_pool(name="work", bufs=3) as work,
            tc.tile_pool(name="wts", bufs=4) as wts,
            tc.tile_pool(name="ps_t", bufs=2, space="PSUM") as ps_t,
            tc.tile_pool(name="ps_m", bufs=3, space="PSUM") as ps_m,
        ):
            wt_sb = consts.tile([128, 24], F32)
            nc.sync.dma_start(wt_sb[:, :], wt[:, :])
            bt_sb = consts.tile([128, 24], F32)
            nc.sync.dma_start(bt_sb[:, :], bt[:, :])
            id_sb = consts.tile([128, 128], F16)
            nc.sync.dma_start(id_sb[:, :], ident[:, :])
            l3_sb = consts.tile([128, 2, 160], F16)
            nc.sync.dma_start(l3_sb[:, :, :], l3p[:, :, :])

            for st in range(N_ST):
                base = st * ST_ROWS
                # partition p <- rows base + p*G + q (q = 0..G-1), contiguous per partition
                xs = xc[base:base + ST_ROWS, :].rearrange("(p g) f -> p g f", g=G)
                x_sb = io.tile([128, G, F], F32, tag="x")
                nc.sync.dma_start(x_sb[:, :, :], xs)

                # z[p,g,f,k] = x[p,g,f] * (w[k]/T) + (b[k]/T)
                z = work.tile([128, G, F, NB], F32, tag="z")
                x_b = x_sb[:, :, :].unsqueeze(3).broadcast_to((128, G, F, NB))
                wt_b = (
                    wt_sb[:, :]
                    .rearrange("p (f k) -> p f k", k=NB)
                    .unsqueeze(1)
                    .broadcast_to((128, G, F, NB))
                )
                bt_b = (
                    bt_sb[:, :]
                    .rearrange("p (f k) -> p f k", k=NB)
                    .unsqueeze(1)
                    .broadcast_to((128, G, F, NB))
                )
                nc.gpsimd.tensor_mul(z[:, :, :, :], x_b, wt_b)
                nc.gpsimd.tensor_add(z[:, :, :, :], z[:, :, :, :], bt_b)

                # sigma = sigmoid(z)   [128, G, 24]
                sig = work.tile([128, G, F * NB], F32, tag="sig")
                nc.scalar.activation(
                    sig[:, :, :].rearrange("p g (f k) -> p g f k", k=NB),
                    z[:, :, :, :],
                    mybir.ActivationFunctionType.Sigmoid,
                )

                def _kron16(dst, c0, c1, eng=nc.vector):
                    in0 = (
                        sig[:, :, c0:c0 + NB]
                        .unsqueeze(3)
                        .broadcast_to((128, G, NB, NB))
                    )
                    in1 = (
                        sig[:, :, c1:c1 + NB]
                        .unsqueeze(2)
                        .broadcast_to((128, G, NB, NB))
                    )
                    eng.tensor_mul(dst, in0, in1)

                def _pair_cols(c0, tag, eng=nc.vector):
                    # sp[p,g, j*2+t] = sig[p,g, c0+j]  (duplicated pairs)
                    spdt = F16 if ACC_MODE == "fast" else F32
                    sp = work.tile([128, G, NB, 2], spdt, tag=tag)
                    eng.tensor_copy(
                        sp[:, :, :, :],
                        sig[:, :, c0:c0 + NB].unsqueeze(3)
                           .broadcast_to((128, G, NB, 2)),
                    )
                    return sp

                def _kron16_paired(dst, c0, sp, eng=nc.vector):
                    # dst[p,g,i,(j,t)] = s[c0+i] * sp[(j,t)] -- 3 free dims
                    in0 = (
                        sig[:, :, c0:c0 + NB]
                        .unsqueeze(3)
                        .broadcast_to((128, G, NB, 2 * NB))
                    )
                    in1 = (
                        sp[:, :, :, :].rearrange("p g j t -> p g (j t)")
                        .unsqueeze(2)
                        .broadcast_to((128, G, NB, 2 * NB))
                    )
                    eng.tensor_mul(dst, in0, in1)

                if ACC_MODE == "acc":
                    a_sb = work.tile([128, G, 16], F32, tag="A")
                    _kron16(a_sb[:, :, :].rearrange("p g (i j) -> p g i j", j=NB), 0, 4)
                else:
                    # A duplicated x2: ap2[p,g, a*2+t] = s0[k0]*s1[k1], a=k0*4+k1
                    s1p = _pair_cols(4, "s1p")
                    ap2 = work.tile([128, G, 16, 2], F16, tag="A")
                    _kron16_paired(
                        ap2[:, :, :, :].rearrange("p g (i j) t -> p g i (j t)", j=NB), 0, s1p)
                u_sb = work.tile([128, G, 16], F16, tag="u")
                _kron16(u_sb[:, :, :].rearrange("p g (i j) -> p g i j", j=NB), 8, 12,
                        eng=nc.gpsimd)
                # v duplicated x2: vp2[p,g, vv*2+t] = s4[k4]*s5[k5], vv=k4*4+k5
                s5p = _pair_cols(20, "s5p", eng=nc.gpsimd)
                vp2 = work.tile([128, G, 16, 2], F16, tag="v")
                _kron16_paired(
                    vp2[:, :, :, :].rearrange("p g (i j) t -> p g i (j t)", j=NB), 16, s5p,
                    eng=nc.gpsimd)

                # Bm[p,g, vv*16+uu] = u[p,g,uu] * v[p,g,vv]   [128, G, 256]
                # (column order vv-major; folded into the host L3 layout).
                # Per-q ops keep APs at 3 free dims; innermost [1,2] fp16
                # pairs put the DVE in its 2x mode.
                bm = work.tile([128, G, 256], F16, tag="bm")
                for q in range(G):
                    nc.vector.tensor_mul(
                        bm[:, q, :].rearrange("p (i j t) -> p i j t", j=8, t=2),
                        u_sb[:, q, :].rearrange("p (j t) -> p j t", t=2)
                            .unsqueeze(1).broadcast_to((128, 16, 8, 2)),
                        vp2[:, q, :, :].unsqueeze(2).broadcast_to((128, 16, 8, 2)),
                    )

                # M[p, q, a*10+c] accumulated over the 256-contraction.
                # Two half-supertile PSUM tiles so matmuls of the next group
                # can start while the previous group is being copied out.
                if ACC_MODE != "acc":
                    msb = work.tile([128, G, 160], F16, tag="msb")
                else:
                    prod32 = work.tile([128, G, 160], F32, tag="msb")
                for m in range(G // 4):
                    tp = ps_t.tile([128, 4, 256], F16, tag="tp")
                    bmt4 = wts.tile([128, 4, 256], F16, tag="bmt")
                    for qq in range(4):
                        q = m * 4 + qq
                        nc.tensor.transpose(
                            tp[:, qq, 0:128], bm[:, q, 0:128], id_sb[:, :])
                        nc.tensor.transpose(
                            tp[:, qq, 128:256], bm[:, q, 128:256], id_sb[:, :])
                    # one batched evacuation per 4 row-groups; uint32 bitcast
                    # halves the element count
                    nc.scalar.copy(
                        bmt4[:, :, :].bitcast(mybir.dt.uint32),
                        tp[:, :, :].bitcast(mybir.dt.uint32),
                    )
                    mps = ps_m.tile([128, 4, 256], F32, tag="m")
                    for qq in range(4):
                        q = m * 4 + qq
                        nc.tensor.matmul(
                            mps[:, qq, 0:160], bmt4[:, qq, 0:128], l3_sb[:, 0, :],
                            start=True, stop=False,
                        )
                        nc.tensor.matmul(
                            mps[:, qq, 0:160], bmt4[:, qq, 128:256], l3_sb[:, 1, :],
                            start=False, stop=True,
                        )
                    if ACC_MODE != "acc":
                        # M -> SBUF fp16 (scalar engine)
                        nc.scalar.copy(
                            msb[:, m * 4:(m + 1) * 4, :], mps[:, :, 0:160])
                    else:
                        # prod in fp32 straight from PSUM
                        nc.vector.tensor_mul(
                            prod32[:, m * 4:(m + 1) * 4, :]
                                .rearrange("p g (a c) -> p g a c", c=NCLS),
                            a_sb[:, m * 4:(m + 1) * 4, :].unsqueeze(3)
                                .broadcast_to((128, 4, 16, NCLS)),
                            mps[:, :, 0:160].rearrange("p g (a c) -> p g a c", c=NCLS),
                        )

                oq = io.tile([128, G, NCLS], F32, tag="oq")
                if ACC_MODE == "acc":
                    nc.vector.tensor_reduce(
                        oq[:, :, :],
                        prod32[:, :, :].rearrange("p g (a c) -> p g c a", c=NCLS),
                        axis=mybir.AxisListType.X,
                        op=mybir.AluOpType.add,
                    )
                else:
                    # prod[p,g, a*10+c] = A[a] * M[a*10+c]; packed pairs -> 2x
                    prod = work.tile([128, G, 160], F16, tag="prod")
                    for q in range(G):
                        nc.vector.tensor_mul(
                            prod[:, q, :].rearrange("p (a cp t) -> p a cp t", cp=5, t=2),
                            ap2[:, q, :, :].unsqueeze(2).broadcast_to((128, 16, 5, 2)),
                            msb[:, q, :].rearrange("p (a cp t) -> p a cp t", cp=5, t=2),
                        )
                    if ACC_MODE == "mid":
                        nc.vector.tensor_reduce(
                            oq[:, :, :],
                            prod[:, :, :].rearrange("p g (a c) -> p g c a", c=NCLS),
                            axis=mybir.AxisListType.X,
                            op=mybir.AluOpType.add,
                        )
                    else:
                        f1 = work.tile([128, G, 80], F16, tag="f1")
                        nc.vector.tensor_add(f1[:, :, :], prod[:, :, 0:80], prod[:, :, 80:160])
                        f2 = work.tile([128, G, 40], F16, tag="f2")
                        nc.vector.tensor_add(f2[:, :, :], f1[:, :, 0:40], f1[:, :, 40:80])
                        nc.vector.tensor_reduce(
                            oq[:, :, :],
                            f2[:, :, :].rearrange("p g (a c) -> p g c a", c=NCLS),
                            axis=mybir.AxisListType.X,
                            op=mybir.AluOpType.add,
                        )

                od = outc[base:base + ST_ROWS, :].rearrange("(p g) c -> p g c", g=G)
                nc.sync.dma_start(od, oq[:, :, :])

    nc.compile()
    return nc


def _host_prep(x, beta, leaves2classes):
    x = np.ascontiguousarray(np.asarray(x, dtype=np.float32))
    beta = np.asarray(beta, dtype=np.float32)
    L = np.asarray(leaves2classes, dtype=np.float32)

    w = np.linspace(1.0, float(NB), NB, dtype=np.float32)
    bs = np.sort(beta)
    b = np.concatenate([np.zeros(1, np.float32), np.cumsum(-bs, dtype=np.float32)])

    wt24 = np.tile(w / np.float32(TEMP), F).astype(np.float32)       # [(f,k)] = w[k]/T
    bt24 = np.tile(b / np.float32(TEMP), F).astype(np.float32)
    WT = np.ascontiguousarray(np.broadcast_to(wt24, (128, 24)))
    BT = np.ascontiguousarray(np.broadcast_to(bt24, (128, 24)))

    # L3[b, a*10+c] = L[a*256+b, c];  then rows permuted to the device's
    # Bm column order j = vv*16+uu  (b_leaf = uu*16+vv)
    L3 = L.reshape(16, 256, NCLS).transpose(1, 0, 2).reshape(256, 16 * NCLS)
    j = np.arange(256)
    L3 = L3[(j % 16) * 16 + (j // 16)]
    L3P = np.ascontiguousarray(L3.reshape(2, 128, 16 * NCLS).transpose(1, 0, 2)).astype(np.float16)

    ident = np.eye(128, dtype=np.float16)
    return x, WT, BT, ident, L3P


def kernel(x, beta, leaves2classes):
    x, WT, BT, ident, L3P = _host_prep(x, beta, leaves2classes)

    if "nc" not in _NC_CACHE:
        _NC_CACHE["nc"] = _build_nc()
    nc = _NC_CACHE["nc"]

    in_maps = []
    for c in range(CORES):
        in_maps.append({
            "xc": np.ascontiguousarray(x[c * ROWS:(c + 1) * ROWS]),
            "wt": WT,
            "bt": BT,
            "ident": ident,
            "l3p": L3P,
        })
    res = run_bass_kernel_spmd(nc, in_maps, core_ids=list(range(CORES)))
    out = np.concatenate([r["outc"] for r in res.results], axis=0)
    return out.astype(np.float32)

